# revision 1
# baseline (speedup 1.0000x reference)
"""Trainium2 Bass kernel for the e3nn-style equivariant 3D convolution.

Strategy:
  * The whole module (self-connection linear + radial-weight kernel
    generation + 5x5x5 conv, y = sc + 0.1*conv) collapses into ONE 3D
    convolution: the self-connection is a 1x1x1 conv, folded into the
    center tap of the 5x5x5 kernel. The tiny (5^3 x 64 x 64) kernel is
    built on host in numpy.
  * Data-parallel over the X axis across 8 NeuronCores: each core gets a
    12-plane input slab (8 output planes + 2-halo each side, host-padded)
    and produces an 8-plane output slab.
  * Per core, the conv is an implicit GEMM on the tensor engine with
    fp32r (fp22-mantissa, full-rate) matmuls:
      - SBUF x layout: per x-plane tile [128, 34, 68]: partition = channel
        + 64*(y parity), free = (y-pair row, z) with 2-voxel zero pads.
      - contraction K = 128 = 64 channels x 2 y-parities,
        stationary M = 128 = 64 out-channels x 2 output y-parities,
        moving N = 512 = 8 y-pair rows x 64 z.
      - All 125 taps of one output tile accumulate in a single PSUM bank
        via 75 matmuls (dx, dz, y-shift sigma in {-1,0,1}); dy is encoded
        in the 2x2 (parity x parity) block structure of each weight.
  * PSUM -> SBUF copy on the vector engine, DMA out, host concatenates.
"""

import math
import numpy as np

import concourse.bass as bass
import concourse.bacc as bacc
import concourse.mybir as mybir
from concourse import tile as tile_mod
from concourse.bass_utils import run_bass_kernel_spmd

# ---------------------------------------------------------------- constants
SIZE = 5
MUL = 16
DIM = 4 * MUL                  # 64 channels
INV_SQRT3 = 1.0 / math.sqrt(3.0)
ALPHA_0 = math.sqrt(1.0 / (2 * MUL))
ALPHA_1 = math.sqrt(3.0 / (2 * MUL))

B, C, X, Y, Z = 2, 64, 64, 64, 64
NCORE = 8
XO = X // NCORE                # output x-planes per core
XP = XO + 4                    # input x-planes per core (2-halo)
R = 68                         # padded z row width
YPR = 34                       # y-pair rows per parity (incl. 2 pad rows)
NBLK = 75                      # 5 dx * 5 dz * 3 sigma weight blocks
NSLOT = 7                      # rotating x-plane SBUF slots

F32 = mybir.dt.float32
F32R = mybir.dt.float32r


# ------------------------------------------------------- host-side weights
def _build_conv_weights(lin_w0, lin_w1, tp_weight):
    """Full folded conv kernel K_oi (64, 64, 5, 5, 5) fp64:
    y[b,o,x,y,z] = sum_{i,t} K_oi[o,i,tx,ty,tz] * x[b,i,x+tx-2,y+ty-2,z+tz-2]
    """
    r = np.linspace(-1.0, 1.0, SIZE, dtype=np.float64)
    gx, gy, gz = np.meshgrid(r, r, r, indexing='ij')
    lat = np.stack([gx, gy, gz], axis=-1)
    d = np.linalg.norm(lat, axis=-1)
    unit = np.where(d[..., None] > 0, lat / np.maximum(d[..., None], 1e-12), 0.0)
    sh0 = np.ones_like(d)
    sh1 = math.sqrt(3.0) * unit[..., [1, 2, 0]]

    sigma = 1.0 / (SIZE - 1)
    values = np.linspace(0.0, 1.0, SIZE)
    emb = np.exp(-(((d[..., None] - values) / sigma) ** 2)) / 1.12
    emb = emb @ tp_weight.astype(np.float64)
    emb = emb * (np.cos(math.pi * d) / SIZE ** 1.5)[..., None]
    mm = MUL * MUL
    g = (SIZE, SIZE, SIZE)
    wA = emb[..., 0*mm:1*mm].reshape(*g, MUL, MUL)
    wB = emb[..., 1*mm:2*mm].reshape(*g, MUL, MUL)
    wC = emb[..., 2*mm:3*mm].reshape(*g, MUL, MUL)
    wD = emb[..., 3*mm:4*mm].reshape(*g, MUL, MUL)

    eye3 = np.eye(3)
    k00 = ALPHA_0 * wA * sh0[..., None, None]
    k10 = (ALPHA_0 * INV_SQRT3) * np.einsum('...uw,...i->...uiw', wD, sh1)
    k01 = (ALPHA_1 * INV_SQRT3) * np.einsum('...uw,...k->...uwk', wB, sh1)
    k11 = (ALPHA_1 * INV_SQRT3) * np.einsum('...uw,ik->...uiwk',
                                            wC * sh0[..., None, None], eye3)
    top = np.concatenate([k00, k01.reshape(*g, MUL, 3*MUL)], axis=-1)
    bot = np.concatenate([k10.reshape(*g, 3*MUL, MUL),
                          k11.reshape(*g, 3*MUL, 3*MUL)], axis=-1)
    kernel = np.concatenate([top, bot], axis=-2)          # (5,5,5,in,out)
    K_oi = 0.1 * np.transpose(kernel, (4, 3, 0, 1, 2))    # (o,i,tx,ty,tz)

    # fold self connection (per-irrep linear) into the center tap
    inv = 1.0 / math.sqrt(MUL)
    L = np.zeros((DIM, DIM))
    L[:MUL, :MUL] = lin_w0.T.astype(np.float64) * inv
    iu = np.arange(MUL)
    for iv in range(3):
        L[(MUL + 3*iu[:, None] + iv), (MUL + 3*iu[None, :] + iv)] = \
            lin_w1.T.astype(np.float64) * inv
    K_oi[:, :, 2, 2, 2] += L
    return K_oi


def _pack_weight_blocks(K_oi):
    """(o,i,tx,ty,tz) -> (128, NBLK*128) fp32 stationary blocks.

    Block index bi = (dx*5 + dz)*3 + s, s = sigma+1.
    lhsT[c + 64*par, o + 64*q] = K_oi[o, c, dx, 2*sigma+par-q+2, dz].
    """
    W = np.zeros((128, NBLK * 128), dtype=np.float32)
    for dx in range(5):
        for dz in range(5):
            for s in range(3):
                sig = s - 1
                bi = (dx * 5 + dz) * 3 + s
                for par in range(2):
                    for q in range(2):
                        dy = 2 * sig + par - q + 2
                        if 0 <= dy < 5:
                            W[64*par:64*par+64,
                              bi*128 + 64*q: bi*128 + 64*q + 64] = \
                                K_oi[:, :, dx, dy, dz].T
    return W


# ------------------------------------------------------------- bass kernel
def _emit(tc, nc, x_ap, w_ap, y_ap, reps=1):
    # x_ap: [B, XP, 128, YPR, R] host-packed slot layout (pads included)
    yv = y_ap.rearrange("b o xo (yh par) z -> b o xo par yh z", par=2)

    with (
        tc.tile_pool(name="wpool", bufs=1) as wp,
        tc.tile_pool(name="xpool", bufs=1) as xp,
        tc.tile_pool(name="opool", bufs=4) as op,
        tc.tile_pool(name="pspool", bufs=6, space="PSUM") as pp,
        tc.tile_pool(name="scratchpool", bufs=1, space="PSUM") as scp,
    ):
        wt = wp.tile([128, NBLK, 128], F32R, tag="w")
        nc.sync.dma_start(out=wt[:, :, :], in_=w_ap.rearrange(
            "p (nb m) -> p nb m", m=128))

        slots = [xp.tile([128, YPR, R], F32R, tag=f"xs{i}", name=f"xs{i}")
                 for i in range(NSLOT)]

        # PE "observe" dummies: a 1-column matmul whose only dependency is a
        # just-issued DMA. The fp32r matmul encoding only supports ONE sync
        # wait, so every real matmul must reach walrus with <=1 wait; these
        # dummies advance PE's vector clock past fresh DMA ticks so real
        # group-start matmuls only ever wait on the PSUM-release (DVE) sem.
        scratch = scp.tile([128, 2], F32, tag="scratch", name="scratch")

        def observe(rhs_ap):
            # N=2: fp32r matmuls require an even innermost count on both the
            # moving operand and the PSUM destination.
            nc.tensor.matmul(scratch[:, :], wt[:, 0, :], rhs_ap,
                             start=True, stop=True)

        def load_plane(g, b, p):
            s = slots[g % NSLOT]
            nc.gpsimd.dma_start(out=s[:, :, :], in_=x_ap[b, p])
            return s

        first = True
        for rb in range(reps * B):
            rep, b = divmod(rb, B)
            pre = [load_plane(rb * XP + p, b, p) for p in range(4)]
            if first:
                observe(wt[:, 0, 0:2])
                first = False
            for s_t in pre:
                observe(s_t[:, 0, 0:2])
            for xo in range(XO):
                load_plane(rb * XP + xo + 4, b, xo + 4)
                planes = [slots[(rb * XP + xo + d) % NSLOT] for d in range(5)]
                for t in range(4):
                    yp0 = 8 * t + 1
                    pt = pp.tile([128, 8, 64], F32, tag="ps", name="pt")
                    k = 0
                    for dx in range(5):
                        s_t = planes[dx]
                        for dz in range(5):
                            for s in range(3):
                                sig = s - 1
                                bi = (dx * 5 + dz) * 3 + s
                                nc.tensor.matmul(
                                    pt[:, :, :],
                                    wt[:, bi, :],
                                    s_t[:, yp0+sig: yp0+sig+8, dz: dz+64],
                                    start=(k == 0),
                                    stop=(k == NBLK - 1),
                                )
                                k += 1
                    ot = op.tile([128, 8, 64], F32, tag="ot", name="ot")
                    nc.vector.tensor_copy(out=ot[:, :, :], in_=pt[:, :, :])
                    for q in range(2):
                        nc.gpsimd.dma_start(
                            out=yv[b, :, xo, q, 8*t: 8*t+8, :],
                            in_=ot[64*q: 64*q+64, :, :],
                        )


_CACHED_NC = None


def _build_nc():
    global _CACHED_NC
    if _CACHED_NC is not None:
        return _CACHED_NC
    nc = bacc.Bacc("TRN2", target_bir_lowering=False, debug=False,
                   num_swdge_queues=4)
    x_d = nc.dram_tensor("x", [B, XP, 128, YPR, R], F32R, kind="ExternalInput")
    w_d = nc.dram_tensor("w", [128, NBLK * 128], F32R, kind="ExternalInput")
    y_d = nc.dram_tensor("y", [B, C, XO, Y, Z], F32, kind="ExternalOutput")
    with tile_mod.TileContext(nc) as tc:
        _emit(tc, nc, x_d.ap(), w_d.ap(), y_d.ap())
    nc.compile()
    _CACHED_NC = nc
    return nc


# ---------------------------------------------------------------- entrypoint
TRACE = False          # set True (e.g. from test.py) to capture an NTFF profile
LAST_RESULT = None     # BassKernelResults of the most recent kernel() call


def _pack_shard(sh):
    """(B, C, XP, Y, Z) x-slab -> (B, XP, 128, YPR, R) padded slot layout.

    partition = channel + 64*(y parity); free = (y-pair row, z) with 2-voxel
    zero borders. Row yp holds y = 2*yp - 2 + par.
    """
    out = np.zeros((sh.shape[0], sh.shape[2], 128, YPR, R), dtype=np.float32)
    out[:, :, :64, 1:33, 2:66] = sh[:, :, :, 0::2, :].transpose(0, 2, 1, 3, 4)
    out[:, :, 64:, 1:33, 2:66] = sh[:, :, :, 1::2, :].transpose(0, 2, 1, 3, 4)
    return out


def kernel(x, lin_w0, lin_w1, tp_weight):
    x = np.ascontiguousarray(np.asarray(x, dtype=np.float32))
    K_oi = _build_conv_weights(np.asarray(lin_w0), np.asarray(lin_w1),
                               np.asarray(tp_weight))
    W = _pack_weight_blocks(K_oi)

    xpad = np.pad(x, ((0, 0), (0, 0), (2, 2), (0, 0), (0, 0)))
    in_maps = []
    for i in range(NCORE):
        in_maps.append({
            "x": _pack_shard(xpad[:, :, i*XO: i*XO + XP]),
            "w": W,
        })

    nc = _build_nc()
    kwargs = {}
    if TRACE:
        kwargs = {"trace": True, "trace_cores": list(range(NCORE))}
    res = run_bass_kernel_spmd(nc, in_maps, list(range(NCORE)), **kwargs)
    global LAST_RESULT
    LAST_RESULT = res
    out = np.concatenate([res.results[i]["y"] for i in range(NCORE)], axis=2)
    return out.astype(np.float32)



# revision 7
# speedup vs baseline: 4.1405x; 4.1405x over previous
"""Trainium2 Bass kernel for the e3nn-style equivariant 3D convolution.

Strategy (v2 — fp8 DoubleRow):
  * The whole module (self-connection linear + radial-weight kernel
    generation + 5x5x5 conv, y = sc + 0.1*conv) collapses into ONE 3D
    convolution: the self-connection is a 1x1x1 conv folded into the
    center tap. The tiny (5^3 x 64 x 64) kernel is built on host.
  * Data-parallel over X across 8 NeuronCores (8 output planes + 2-halo
    input slab per core, host-padded).
  * Implicit GEMM on the tensor engine, fp8(e4m3) with DoubleRow perf
    mode: each matmul contracts TWO K=128 blocks (K=256) at 0.5
    cycles/output-column — 4x the per-block rate of fp32r.
  * Precision: the radial envelope concentrates ~99.99% of kernel energy
    in ~45 of the 125 taps. Per-run, blocks (dx,dz,sigma) are ranked by
    energy; low-energy blocks are dropped (error ~ sqrt(dropped energy)).
    fp8 quantization error is compensated with a correction pass:
      out = Wh.xh  +  (1/64) * [ (64*Wres).xh + Wh.(64*xres) ]
    where Wh=e4m3(W), Wres=W-Wh, xh=e4m3(x), xres=x-xh (residuals are
    stored x64 to stay out of e4m3's subnormal range). Each correction
    block fits ONE DoubleRow matmul (k-tile0: scaled W-residual vs xh,
    k-tile1: Wh vs scaled x-residual). The two PSUM accumulators are
    combined by a single DVE scalar_tensor_tensor op per tile.
  * SBUF x layout: one tile [128, NSLOT, 2, YPR, R]; partition = channel
    + 64*(y parity); free = (plane slot, hi/lo, y-pair row, z) with
    2-voxel zero pads. All DoubleRow k-tile pairs read within a single
    plane slot, so custom strided APs never alias the prefetch DMA.
"""

import math
import numpy as np
import ml_dtypes

import concourse.bass as bass
import concourse.bacc as bacc
import concourse.mybir as mybir
from concourse import tile as tile_mod
from concourse.bass import AP
from concourse.bass_utils import run_bass_kernel_spmd

# ---------------------------------------------------------------- constants
SIZE = 5
MUL = 16
DIM = 4 * MUL                  # 64 channels
INV_SQRT3 = 1.0 / math.sqrt(3.0)
ALPHA_0 = math.sqrt(1.0 / (2 * MUL))
ALPHA_1 = math.sqrt(3.0 / (2 * MUL))

B, C, X, Y, Z = 2, 64, 64, 64, 64
NCORE = 8
XO = X // NCORE                # output x-planes per core
XP = XO + 4                    # input x-planes per core (2-halo)
R = 68                         # padded z row width
YPR = 34                       # y-pair rows per parity (incl. 2 pad rows)
NSLOT = 7                      # rotating x-plane SBUF slots

SRES = 64.0                    # scale for both W and x residuals
WSCALE = 16.0                  # pre-scale for W: |W|~0.01 rms sits in e4m3's
                               # subnormal range; x16 moves it to normal range
HH_RESID = 1.0e-4              # kept-block energy residual for the main pass
CORR_RESID = 1.5e-3            # residual threshold for the correction pass

F32 = mybir.dt.float32
F8 = mybir.dt.float8e4
E4 = ml_dtypes.float8_e4m3

SLOT_ST = 2 * YPR * R          # fp8 elements per plane slot per partition
HILO_ST = YPR * R


# ------------------------------------------------------- host-side weights
def _build_conv_weights(lin_w0, lin_w1, tp_weight):
    """Full folded conv kernel K_oi (64, 64, 5, 5, 5) fp64:
    y[b,o,x,y,z] = sum_{i,t} K_oi[o,i,tx,ty,tz] * x[b,i,x+tx-2,y+ty-2,z+tz-2]
    """
    r = np.linspace(-1.0, 1.0, SIZE, dtype=np.float64)
    gx, gy, gz = np.meshgrid(r, r, r, indexing='ij')
    lat = np.stack([gx, gy, gz], axis=-1)
    d = np.linalg.norm(lat, axis=-1)
    unit = np.where(d[..., None] > 0, lat / np.maximum(d[..., None], 1e-12), 0.0)
    sh0 = np.ones_like(d)
    sh1 = math.sqrt(3.0) * unit[..., [1, 2, 0]]

    sigma = 1.0 / (SIZE - 1)
    values = np.linspace(0.0, 1.0, SIZE)
    emb = np.exp(-(((d[..., None] - values) / sigma) ** 2)) / 1.12
    emb = emb @ tp_weight.astype(np.float64)
    emb = emb * (np.cos(math.pi * d) / SIZE ** 1.5)[..., None]
    mm = MUL * MUL
    g = (SIZE, SIZE, SIZE)
    wA = emb[..., 0*mm:1*mm].reshape(*g, MUL, MUL)
    wB = emb[..., 1*mm:2*mm].reshape(*g, MUL, MUL)
    wC = emb[..., 2*mm:3*mm].reshape(*g, MUL, MUL)
    wD = emb[..., 3*mm:4*mm].reshape(*g, MUL, MUL)

    eye3 = np.eye(3)
    k00 = ALPHA_0 * wA * sh0[..., None, None]
    k10 = (ALPHA_0 * INV_SQRT3) * np.einsum('...uw,...i->...uiw', wD, sh1)
    k01 = (ALPHA_1 * INV_SQRT3) * np.einsum('...uw,...k->...uwk', wB, sh1)
    k11 = (ALPHA_1 * INV_SQRT3) * np.einsum('...uw,ik->...uiwk',
                                            wC * sh0[..., None, None], eye3)
    top = np.concatenate([k00, k01.reshape(*g, MUL, 3*MUL)], axis=-1)
    bot = np.concatenate([k10.reshape(*g, 3*MUL, MUL),
                          k11.reshape(*g, 3*MUL, 3*MUL)], axis=-1)
    kernel = np.concatenate([top, bot], axis=-2)          # (5,5,5,in,out)
    K_oi = 0.1 * np.transpose(kernel, (4, 3, 0, 1, 2))    # (o,i,tx,ty,tz)

    # fold self connection (per-irrep linear) into the center tap
    inv = 1.0 / math.sqrt(MUL)
    L = np.zeros((DIM, DIM))
    L[:MUL, :MUL] = lin_w0.T.astype(np.float64) * inv
    iu = np.arange(MUL)
    for iv in range(3):
        L[(MUL + 3*iu[:, None] + iv), (MUL + 3*iu[None, :] + iv)] = \
            lin_w1.T.astype(np.float64) * inv
    K_oi[:, :, 2, 2, 2] += L
    return K_oi


def _block_dys(s):
    """(par, q) -> dy map of block sigma index s (sigma = s-1)."""
    out = {}
    for par in range(2):
        for q in range(2):
            dy = 2 * (s - 1) + par - q + 2
            if 0 <= dy < 5:
                out[(par, q)] = dy
    return out


def _select_blocks(K_oi):
    """Rank (dx, dz, s) blocks by covered tap energy; return
    (hh_pairs, corr_blocks). hh_pairs is a list of (blockA, blockB|None)
    with both blocks on the same dx plane."""
    E = np.sum(K_oi.astype(np.float64) ** 2, axis=(0, 1))   # (5,5,5)
    tot = E.sum()
    blocks = []
    for dx in range(5):
        for dz in range(5):
            for s in range(3):
                e = sum(0.5 * E[dx, dy, dz]
                        for dy in _block_dys(s).values()
                        for _ in [0])
                # each (par,q) slot covers its dy once; weight 0.5 per q
                e = 0.0
                for (par, q), dy in _block_dys(s).items():
                    e += 0.5 * E[dx, dy, dz]
                blocks.append((e / tot, dx, dz, s))
    blocks.sort(reverse=True)

    def pick(resid_target):
        out, resid = [], 1.0
        for e, dx, dz, s in blocks:
            if resid <= resid_target:
                break
            out.append((dx, dz, s))
            resid -= e
        return out

    hh = pick(HH_RESID)
    corr = pick(CORR_RESID)
    # corr must be a subset of hh (it corrects quantization of kept blocks)
    assert len(corr) <= len(hh)

    # pair hh blocks within each dx plane
    hh_pairs = []
    for dx in range(5):
        plane = [b for b in hh if b[0] == dx]
        for i in range(0, len(plane) - 1, 2):
            hh_pairs.append((plane[i], plane[i + 1]))
        if len(plane) % 2:
            hh_pairs.append((plane[-1], None))
    corr.sort()
    return hh_pairs, corr


def _ktile(Ksrc, s):
    """[128,128] stationary k-tile from Ksrc (o,i,dy) stack for sigma idx s:
    T[c + 64*par, o + 64*q] = Ksrc[o, c, dy(par,q)]."""
    T = np.zeros((128, 128), dtype=np.float32)
    for (par, q), dy in _block_dys(s).items():
        T[64*par:64*par+64, 64*q:64*q+64] = Ksrc[:, :, dy].T
    return T


def _pack_weights(K_oi, hh_pairs, corr_blocks):
    """Build fp8 stationary tensors:
    w_hh [128, NPAIR, 2, 128] (both k-tiles from Wh),
    w_cr [128, NCORR, 2, 128] (k-tile0: 64*Wres vs xh, k-tile1: Wh vs 64*xres).
    """
    Ws = np.asarray(WSCALE * K_oi, np.float32)
    Wh = Ws.astype(E4).astype(np.float32)
    Wres = (SRES * (Ws - Wh)).astype(np.float32)

    def hh_tile(blk):
        if blk is None:
            return np.zeros((128, 128), np.float32)
        dx, dz, s = blk
        return _ktile(Wh[:, :, dx, :, dz], s)

    w_hh = np.zeros((128, len(hh_pairs), 2, 128), np.float32)
    for i, (a, b) in enumerate(hh_pairs):
        w_hh[:, i, 0, :] = hh_tile(a)
        w_hh[:, i, 1, :] = hh_tile(b)

    w_cr = np.zeros((128, len(corr_blocks), 2, 128), np.float32)
    for j, (dx, dz, s) in enumerate(corr_blocks):
        w_cr[:, j, 0, :] = _ktile(Wres[:, :, dx, :, dz], s)
        w_cr[:, j, 1, :] = _ktile(Wh[:, :, dx, :, dz], s)
    return w_hh.astype(E4), w_cr.astype(E4)


# ------------------------------------------------------------- bass kernel
def _emit(tc, nc, x_ap, whh_ap, wcr_ap, y_ap, hh_pairs, corr_blocks):
    npair, ncorr = len(hh_pairs), len(corr_blocks)
    yv = y_ap.rearrange("b o xo (yh par) z -> b o xo par yh z", par=2)

    with (
        tc.tile_pool(name="wpool", bufs=1) as wp,
        tc.tile_pool(name="xpool", bufs=1) as xp,
        tc.tile_pool(name="opool", bufs=4) as op,
        tc.tile_pool(name="cpool", bufs=3) as cp,
        tc.tile_pool(name="pspool_a", bufs=3, space="PSUM") as ppa,
        tc.tile_pool(name="pspool_b", bufs=3, space="PSUM") as ppb,
        tc.tile_pool(name="scratchpool", bufs=1, space="PSUM") as scp,
    ):
        wh_t = wp.tile([128, npair, 2, 128], F8, tag="whh")
        wc_t = wp.tile([128, ncorr, 2, 128], F8, tag="wcr")
        nc.sync.dma_start(out=wh_t[:, :, :, :], in_=whh_ap)
        nc.sync.dma_start(out=wc_t[:, :, :, :], in_=wcr_ap)

        xt = xp.tile([128, NSLOT, 2, YPR, R], F8, tag="xs", name="xs")
        xfull = xt[:, :, :, :, :]
        xap0 = list(xfull.ap[0])
        xbase = xfull.offset

        def xoff(slot, hilo, row, col):
            return slot * SLOT_ST + hilo * HILO_ST + row * R + col

        def rhs_pair(o0, o1):
            return AP(xfull.tensor, xbase + o0,
                      [xap0, [o1 - o0, 2], [R, 8], [1, 64]])

        # PE "observe" dummies: a 2-column matmul whose only dependency is a
        # just-issued DMA. Matmuls reach walrus with <=1 sync wait; these
        # advance PE's vector clock past fresh DMA ticks so real group-start
        # matmuls only ever wait on the PSUM-release (DVE) semaphore.
        scratch = scp.tile([128, 2], F32, tag="scratch", name="scratch")

        def observe(rhs_ap):
            nc.tensor.matmul(scratch[:, :], wh_t[:, 0, 0, :], rhs_ap,
                             start=True, stop=True)

        def load_plane(g, b, p):
            s = g % NSLOT
            nc.gpsimd.dma_start(out=xt[:, s, :, :, :], in_=x_ap[b, p])
            return s

        first = True
        for rb in range(B):
            b = rb
            pre = [load_plane(rb * XP + p, b, p) for p in range(4)]
            if first:
                observe(wh_t[:, 0, 1, 0:2])
                observe(wc_t[:, 0, 0, 0:2])
                first = False
            for s in pre:
                observe(xt[:, s, 0, 0, 0:2])
            for xo in range(XO):
                s_new = load_plane(rb * XP + xo + 4, b, xo + 4)
                slot = [(rb * XP + xo + d) % NSLOT for d in range(5)]
                for t in range(4):
                    yp0 = 8 * t + 1
                    pta = ppa.tile([128, 8, 64], F32, tag="psa", name="pta")
                    ptb = ppb.tile([128, 8, 64], F32, tag="psb", name="ptb")
                    for i, (blka, blkb) in enumerate(hh_pairs):
                        dxa, dza, sa = blka
                        o0 = xoff(slot[dxa], 0, yp0 + sa - 1, dza)
                        if blkb is None:
                            row = yp0 + sa - 1
                            o1 = o0 + (R if row + 8 <= YPR - 1 else -R)
                        else:
                            dxb, dzb, sb = blkb
                            o1 = xoff(slot[dxb], 0, yp0 + sb - 1, dzb)
                        nc.tensor.matmul(
                            pta[:, :, :], wh_t[:, i, :, :], rhs_pair(o0, o1),
                            start=(i == 0), stop=(i == npair - 1),
                            perf_mode=mybir.MatmulPerfMode.DoubleRow)
                    for j, (dx, dz, s) in enumerate(corr_blocks):
                        o0 = xoff(slot[dx], 0, yp0 + s - 1, dz)
                        nc.tensor.matmul(
                            ptb[:, :, :], wc_t[:, j, :, :],
                            rhs_pair(o0, o0 + HILO_ST),
                            start=(j == 0), stop=(j == ncorr - 1),
                            perf_mode=mybir.MatmulPerfMode.DoubleRow)
                    # PSUM combine: only one PSUM input allowed per DVE op, so
                    # ACT scales the correction bank to SBUF, DVE adds bank A.
                    ct = cp.tile([128, 8, 64], F32, tag="ct", name="ct")
                    nc.scalar.activation(
                        out=ct[:, :, :], in_=ptb[:, :, :],
                        func=mybir.ActivationFunctionType.Copy,
                        scale=1.0 / (SRES * WSCALE))
                    ot = op.tile([128, 8, 64], F32, tag="ot", name="ot")
                    nc.vector.scalar_tensor_tensor(
                        out=ot[:, :, :], in0=pta[:, :, :], scalar=1.0 / WSCALE,
                        in1=ct[:, :, :], op0=mybir.AluOpType.mult,
                        op1=mybir.AluOpType.add)
                    for q in range(2):
                        nc.gpsimd.dma_start(
                            out=yv[b, :, xo, q, 8*t: 8*t+8, :],
                            in_=ot[64*q: 64*q+64, :, :],
                        )


_CACHED = {}


def _build_nc(hh_pairs, corr_blocks):
    key = (tuple(hh_pairs), tuple(corr_blocks))
    if key in _CACHED:
        return _CACHED[key]
    nc = bacc.Bacc("TRN2", target_bir_lowering=False, debug=False,
                   num_swdge_queues=4)
    x_d = nc.dram_tensor("x", [B, XP, 128, 2, YPR, R], F8,
                         kind="ExternalInput")
    wh_d = nc.dram_tensor("w_hh", [128, len(hh_pairs), 2, 128], F8,
                          kind="ExternalInput")
    wc_d = nc.dram_tensor("w_cr", [128, len(corr_blocks), 2, 128], F8,
                          kind="ExternalInput")
    y_d = nc.dram_tensor("y", [B, C, XO, Y, Z], F32, kind="ExternalOutput")
    with tile_mod.TileContext(nc) as tc:
        _emit(tc, nc, x_d.ap(), wh_d.ap(), wc_d.ap(), y_d.ap(),
              hh_pairs, corr_blocks)
    nc.compile()
    _CACHED[key] = nc
    return nc


# ---------------------------------------------------------------- entrypoint
TRACE = False          # set True (e.g. from test.py) to capture an NTFF profile
LAST_RESULT = None     # BassKernelResults of the most recent kernel() call
LAST_NC = None


def _pack_shard(xh, xl):
    """(B, C, XP, Y, Z) fp8 slabs -> (B, XP, 128, 2, YPR, R) slot layout.

    partition = channel + 64*(y parity); free = (hi/lo, y-pair row, z) with
    zero borders. Row yp holds y = 2*yp - 2 + par.
    """
    out = np.zeros((xh.shape[0], xh.shape[2], 128, 2, YPR, R), dtype=E4)
    for h, src in enumerate((xh, xl)):
        out[:, :, :64, h, 1:33, 2:66] = src[:, :, :, 0::2, :].transpose(0, 2, 1, 3, 4)
        out[:, :, 64:, h, 1:33, 2:66] = src[:, :, :, 1::2, :].transpose(0, 2, 1, 3, 4)
    return out


def kernel(x, lin_w0, lin_w1, tp_weight):
    x = np.ascontiguousarray(np.asarray(x, dtype=np.float32))
    K_oi = _build_conv_weights(np.asarray(lin_w0), np.asarray(lin_w1),
                               np.asarray(tp_weight))
    hh_pairs, corr_blocks = _select_blocks(K_oi)
    whh, wcr = _pack_weights(K_oi, hh_pairs, corr_blocks)

    xh8 = x.astype(E4)
    xl8 = (SRES * (x - xh8.astype(np.float32))).astype(E4)
    xh8 = np.pad(xh8, ((0, 0), (0, 0), (2, 2), (0, 0), (0, 0)))
    xl8 = np.pad(xl8, ((0, 0), (0, 0), (2, 2), (0, 0), (0, 0)))

    in_maps = []
    for i in range(NCORE):
        in_maps.append({
            "x": _pack_shard(xh8[:, :, i*XO: i*XO + XP],
                             xl8[:, :, i*XO: i*XO + XP]),
            "w_hh": whh,
            "w_cr": wcr,
        })

    nc = _build_nc(hh_pairs, corr_blocks)
    global LAST_NC, LAST_RESULT
    LAST_NC = nc
    kwargs = {}
    if TRACE:
        kwargs = {"trace": True, "trace_cores": list(range(NCORE))}
    res = run_bass_kernel_spmd(nc, in_maps, list(range(NCORE)), **kwargs)
    LAST_RESULT = res
    out = np.concatenate([res.results[i]["y"] for i in range(NCORE)], axis=2)
    return out.astype(np.float32)
